# revision 15
# baseline (speedup 1.0000x reference)
"""AttentiveFPConv GNN message-passing kernel for 8 Trainium2 NeuronCores.

Reference computation (all fp32):
    alpha = sigmoid(x[col] @ Wa_w + Wa_b)          # per-edge attention
    neigh = x[col] * alpha                          # per-edge message
    aggr  = segment_sum(neigh, row, N)              # per-node aggregation
    out   = tanh(x @ Wn_w + Wn_b + aggr @ Wg_w + Wg_b)

Key algebraic identity: alpha depends only on the source node, so
    h = x * sigmoid(x @ Wa_w + Wa_b)                # per-NODE tensor
    aggr[n] = sum_{e: row[e]=n} h[col[e]]           # gather + segment-sum

Sharding: destination-node sharding. Core k owns nodes [5000k, 5000(k+1))
and ALL edges targeting them. No collective needed.

Per-core pipeline (everything stays transposed: [feature, node]):
  Phase 1: hT = xT * sigmoid(Wa^T xT) for ALL nodes (replicated);
           PE-transpose each 4992-node piece and write h row-major with a
           PARTITION-MAJOR row permutation so each partition's DMA line is
           contiguous (full-bandwidth writes). h is split into h1 (first
           19968 rows) and h2 so gathers from h1 overlap the second half
           of phase 1. Host remaps gather indices through the permutation.
           PSUM->SBUF transpose copies alternate Scalar/Vector engines.
  Phase 2: dma_gather h[col] in destination-sorted edge order (4 SWDGE
           queues, 1024-index chunks, 32KB descriptor carveout to
           double-buffer each queue's ring). Within each destination
           block the edges are sorted by source h-row for DRAM page
           locality. Pass A consumes h1-sourced edges into aggA (SBUF);
           pass B consumes h2-sourced edges. Segment-sum via one-hot
           matmuls into PSUM aggrT [D, 512] per 4-block group; the
           one-hot M tiles stream from HBM as FP8 (exact for 0/1, half
           the bytes of bf16, and keeps DVE/Pool free of M-building).
  Phase 3 (fused into pass B): poT = Wn^T xT_own + Wg^T (aggA + aggB)
           accumulated in PSUM, outT = tanh(poT + bias) with the
           per-feature bias applied by the activation unit.
           outT [D, 5000] f32 to HBM; host transposes.
"""

import numpy as np
import ml_dtypes

BF16 = ml_dtypes.bfloat16
FP8 = ml_dtypes.float8_e4m3

# ---------------------------------------------------------------- parameters


class P:
    """Problem/kernel parameters (full-size defaults; shrinkable for tests)."""

    def __init__(self, N=40000, D=128, NCORES=8, HSPLIT=19968,
                 GCHUNK=1024, NQ=4, PIECE=4992, MGT=16, SCRATCH=49152):
        assert D == 128
        self.N, self.D, self.NCORES = N, D, NCORES
        self.NB = N // NCORES                 # nodes per core
        self.HSPLIT = HSPLIT                  # h-row split (h1/h2 boundary)
        self.GCHUNK = GCHUNK                  # idxs per dma_gather call
        self.GT = GCHUNK // 128               # gather tiles per chunk
        self.NQ = NQ                          # SWDGE queues for dma_gather
        self.PIECE = PIECE                    # nodes per phase-1 piece
        self.PT = PIECE // 128                # transpose tiles per piece
        self.MGT = MGT                        # one-hot tiles per M-stream load
        self.NBLK = (self.NB + 127) // 128    # 128-node blocks per core
        self.SCRATCH = SCRATCH                # SWDGE descriptor carveout bytes
        assert HSPLIT % PIECE == 0
        assert GCHUNK * 16 <= SCRATCH


def hrow_of_node(p: P, n: np.ndarray) -> np.ndarray:
    """h row index for node n under the partition-major piece layout."""
    n = np.asarray(n, np.int64)
    q = n // p.PIECE
    i = n - q * p.PIECE
    t, pp = i // 128, i % 128
    r = q * p.PIECE + pp * p.PT + t
    full = p.N // p.PIECE            # number of full pieces
    return np.where(q < full, r, n)  # tail nodes keep identity rows


# ------------------------------------------------------------ host edge prep


def prep_edges(p: P, row: np.ndarray, col: np.ndarray):
    """Per-core destination-sorted, block-padded edge streams (A/B split by
    permuted h-row < HSPLIT; block segments sorted by h-row for locality)."""
    row = np.asarray(row).astype(np.int64)
    col = np.asarray(col).astype(np.int64)
    hrow = hrow_of_node(p, col)

    cores = []
    for k in range(p.NCORES):
        sel = (row // p.NB) == k
        r = (row[sel] - k * p.NB).astype(np.int64)
        c = hrow[sel].astype(np.int64)
        order = np.argsort(r, kind="stable")
        r, c = r[order].astype(np.int32), c[order].astype(np.int32)
        lo = np.searchsorted(r, np.arange(p.NBLK) * 128)
        hi = np.searchsorted(r, np.minimum(np.arange(1, p.NBLK + 1) * 128, p.NB))
        blocks = []
        for b in range(p.NBLK):
            rb = r[lo[b]:hi[b]] - b * 128
            cb = c[lo[b]:hi[b]]
            mA = cb < p.HSPLIT
            blocks.append(((cb[mA], rb[mA]), (cb[~mA] - p.HSPLIT, rb[~mA])))
        cores.append(blocks)

    nA = np.array([[len(cores[k][b][0][0]) for b in range(p.NBLK)]
                   for k in range(p.NCORES)])
    nB = np.array([[len(cores[k][b][1][0]) for b in range(p.NBLK)]
                   for k in range(p.NCORES)])
    tA = np.maximum(1, -(-nA.max(axis=0) // 128))          # [NBLK]
    tB = np.maximum(1, -(-nB.max(axis=0) // 128))

    LA, LB = int(tA.sum()) * 128, int(tB.sum()) * 128
    LAg = -(-LA // p.GCHUNK) * p.GCHUNK
    LBg = -(-LB // p.GCHUNK) * p.GCHUNK

    jj = np.arange(128, dtype=np.int32)[None, :]
    per_core = []
    for k in range(p.NCORES):
        idxA = np.zeros(LAg, np.int16); lrA = np.full((LA, 1), -1, np.int32)
        idxB = np.zeros(LBg, np.int16); lrB = np.full((LB, 1), -1, np.int32)
        oA = oB = 0
        for b in range(p.NBLK):
            (cA, rA), (cB, rB) = cores[k][b]
            idxA[oA:oA + len(cA)] = cA; lrA[oA:oA + len(rA), 0] = rA
            oA += int(tA[b]) * 128
            idxB[oB:oB + len(cB)] = cB; lrB[oB:oB + len(rB), 0] = rB
            oB += int(tB[b]) * 128
        # one-hot M streams, fp8 (exact for 0/1), laid out [128, L/128, 128]
        MA = (lrA == jj).reshape(-1, 128, 128).transpose(1, 0, 2)
        MB = (lrB == jj).reshape(-1, 128, 128).transpose(1, 0, 2)
        per_core.append({
            "idxA": np.tile(idxA.reshape(-1, 16).T, (8, 1)),   # [128, LAg/16]
            "idxB": np.tile(idxB.reshape(-1, 16).T, (8, 1)),
            "MA": np.ascontiguousarray(MA).astype(FP8),
            "MB": np.ascontiguousarray(MB).astype(FP8),
        })
    return tA, tB, LA, LB, LAg, LBg, per_core


# ------------------------------------------------------------- device kernel


def build(p: P, tA, tB, LA, LB, LAg, LBg):
    from concourse import bacc, mybir, tile

    f32, bf16, i16 = mybir.dt.float32, mybir.dt.bfloat16, mybir.dt.int16
    fp8 = mybir.dt.float8e4
    AF = mybir.ActivationFunctionType
    nc = bacc.Bacc("TRN2", target_bir_lowering=False, debug=False,
                   num_devices=p.NCORES, num_swdge_queues=p.NQ,
                   dynamic_dma_scratch_size=p.SCRATCH)

    N, D, NB, NBLK = p.N, p.D, p.NB, p.NBLK
    H = p.HSPLIT
    N2 = N - H
    PIECE, PT, MGT = p.PIECE, p.PT, p.MGT
    NFULL = N // PIECE                     # full pieces
    TAIL = N - NFULL * PIECE               # tail nodes (plain rows)

    xT_d = nc.dram_tensor("xT", [D, N], bf16, kind="ExternalInput")
    xTo_d = nc.dram_tensor("xT_own", [D, NB], bf16, kind="ExternalInput")
    WaW_d = nc.dram_tensor("WaW", [D, D], bf16, kind="ExternalInput")
    WnW_d = nc.dram_tensor("WnW", [D, D], bf16, kind="ExternalInput")
    WgW_d = nc.dram_tensor("WgW", [D, D], bf16, kind="ExternalInput")
    WaB_d = nc.dram_tensor("WaB", [D, 1], f32, kind="ExternalInput")
    bias_d = nc.dram_tensor("biasT", [D, 1], f32, kind="ExternalInput")
    ident_d = nc.dram_tensor("ident", [D, D], bf16, kind="ExternalInput")
    idxA_d = nc.dram_tensor("idxA", [128, LAg // 16], i16, kind="ExternalInput")
    idxB_d = nc.dram_tensor("idxB", [128, LBg // 16], i16, kind="ExternalInput")
    MA_d = nc.dram_tensor("MA", [128, LA // 128, D], fp8, kind="ExternalInput")
    MB_d = nc.dram_tensor("MB", [128, LB // 128, D], fp8, kind="ExternalInput")
    outT_d = nc.dram_tensor("outT", [D, NB], f32, kind="ExternalOutput")
    h1_d = nc.dram_tensor("h1", [H, D], bf16, kind="Internal")
    h2_d = nc.dram_tensor("h2", [N2, D], bf16, kind="Internal")

    with tile.TileContext(nc) as tc:
        with (
            tc.tile_pool(name="const", bufs=1) as cpool,
            tc.tile_pool(name="xchunk", bufs=3) as xpool,
            tc.tile_pool(name="hT", bufs=2) as htpool,
            tc.tile_pool(name="hstage", bufs=1) as hspool,
            tc.tile_pool(name="sT", bufs=3) as sTpool,
            tc.tile_pool(name="pg", bufs=2, space="PSUM") as pg_pool,
            tc.tile_pool(name="pt", bufs=2, space="PSUM") as pt_pool,
            tc.tile_pool(name="pa", bufs=2, space="PSUM") as pa_pool,
            tc.tile_pool(name="po", bufs=2, space="PSUM") as po_pool,
            tc.tile_pool(name="sA", bufs=14) as gApool,
            tc.tile_pool(name="sB", bufs=7) as gBpool,
            tc.tile_pool(name="m", bufs=3) as mpool,
            tc.tile_pool(name="aggA", bufs=(NBLK + 3) // 4) as aggApool,
            tc.tile_pool(name="aggB", bufs=2) as aggBpool,
            tc.tile_pool(name="ot", bufs=2) as opool,
            tc.tile_pool(name="tail", bufs=1) as tlpool,
        ):
            # ---- constants into SBUF
            WaW = cpool.tile([D, D], bf16); nc.sync.dma_start(out=WaW[:], in_=WaW_d[:])
            WnW = cpool.tile([D, D], bf16); nc.sync.dma_start(out=WnW[:], in_=WnW_d[:])
            WgW = cpool.tile([D, D], bf16); nc.sync.dma_start(out=WgW[:], in_=WgW_d[:])
            WaB = cpool.tile([D, 1], f32); nc.sync.dma_start(out=WaB[:], in_=WaB_d[:])
            biasT = cpool.tile([D, 1], f32); nc.sync.dma_start(out=biasT[:], in_=bias_d[:])
            ident = cpool.tile([D, D], bf16); nc.sync.dma_start(out=ident[:], in_=ident_d[:])
            xT_own = cpool.tile([D, NB], bf16); nc.sync.dma_start(out=xT_own[:], in_=xTo_d[:])
            idxA_sb = cpool.tile([128, LAg // 16], i16)
            nc.sync.dma_start(out=idxA_sb[:], in_=idxA_d[:])
            idxB_sb = cpool.tile([128, LBg // 16], i16)
            nc.sync.dma_start(out=idxB_sb[:], in_=idxB_d[:])

            # ---- phase 1: hT = xT * sigmoid(Wa^T xT); transpose; h -> HBM
            ncopy = [0]
            for q in range(NFULL + (1 if TAIL else 0)):
                base = q * PIECE
                cn = min(PIECE, N - base)
                h_t = h1_d if base < H else h2_d
                hbase = base if base < H else base - H
                hTp = htpool.tile([D, PIECE], bf16, tag="hT")
                off = 0
                while off < cn:
                    w = min(2048, cn - off)
                    xc = xpool.tile([D, 2048], bf16, tag="xc")
                    nc.sync.dma_start(out=xc[:, :w], in_=xT_d[:, base + off:base + off + w])
                    g0 = 0
                    while g0 < w:
                        gw = min(512, w - g0)
                        pg = pg_pool.tile([D, 512], f32, tag="pg")
                        nc.tensor.matmul(pg[:, :gw], lhsT=WaW[:],
                                         rhs=xc[:, g0:g0 + gw], start=True, stop=True)
                        sT = sTpool.tile([D, 512], bf16, tag="sT")
                        nc.scalar.activation(sT[:, :gw], pg[:, :gw], AF.Sigmoid,
                                             bias=WaB[:, 0:1])
                        nc.vector.tensor_tensor(out=hTp[:, off + g0:off + g0 + gw],
                                                in0=xc[:, g0:g0 + gw],
                                                in1=sT[:, :gw], op=mybir.AluOpType.mult)
                        g0 += gw
                    off += w
                if cn == PIECE:
                    # PE-transpose 128-node tiles; stage partition-major
                    hst = hspool.tile([128, PT, 128], bf16, tag="hst")
                    t0 = 0
                    while t0 < PT:
                        tn = min(4, PT - t0)
                        pt = pt_pool.tile([128, 512], bf16, tag="pt")
                        for qq in range(tn):
                            nc.tensor.transpose(
                                pt[:, qq * 128:(qq + 1) * 128],
                                hTp[:, (t0 + qq) * 128:(t0 + qq + 1) * 128], ident[:])
                        dst = hst[:, t0:t0 + tn, :].rearrange("p t d -> p (t d)")
                        if ncopy[0] % 2 == 0:
                            nc.scalar.activation(dst, pt[:, :tn * 128], AF.Copy)
                        else:
                            nc.vector.tensor_copy(out=dst, in_=pt[:, :tn * 128])
                        ncopy[0] += 1
                        t0 += tn
                    # rows hbase + p*PT + t  <->  hst[p, t, :]  (partition-major)
                    nc.sync.dma_start(
                        out=h_t[hbase:hbase + PIECE, :].rearrange(
                            "(p t) d -> p t d", p=128),
                        in_=hst[:, :, :])
                else:
                    # tail: plain rows base+i
                    pt = pt_pool.tile([128, 512], bf16, tag="pt")
                    nc.tensor.transpose(pt[:cn, :128], hTp[:, :cn], ident[:])
                    tl = tlpool.tile([128, 128], bf16, tag="tl")
                    nc.scalar.activation(tl[:cn, :], pt[:cn, :128], AF.Copy)
                    nc.sync.dma_start(out=h_t[hbase:hbase + cn, :], in_=tl[:cn, :])

            # ---- phase 2+3: gather, one-hot scatter, node-wise linears
            nq_counter = [0]
            gA_tiles = [None] * (LAg // p.GCHUNK)
            gB_tiles = [None] * (LBg // p.GCHUNK)
            mA_tiles = [None] * (-(-(LA // 128) // MGT))
            mB_tiles = [None] * (-(-(LB // 128) // MGT))

            def ensure_chunk(which, ci):
                tiles = gA_tiles if which == "A" else gB_tiles
                if tiles[ci] is not None:
                    return
                g = (gApool if which == "A" else gBpool).tile(
                    [128, p.GT, D], bf16, tag="g" + which)
                idx_sb = idxA_sb if which == "A" else idxB_sb
                src = h1_d[:, :] if which == "A" else h2_d[:, :]
                c0 = ci * (p.GCHUNK // 16)
                nc.gpsimd.dma_gather(
                    out_ap=g[:], in_ap=src, idxs_ap=idx_sb[:, c0:c0 + p.GCHUNK // 16],
                    num_idxs=p.GCHUNK, num_idxs_reg=p.GCHUNK, elem_size=D,
                    queue_num=nq_counter[0] % p.NQ)
                nq_counter[0] += 1
                tiles[ci] = g

            def ensure_mchunk(which, ci):
                tiles = mA_tiles if which == "A" else mB_tiles
                if tiles[ci] is not None:
                    return
                md = MA_d if which == "A" else MB_d
                nt = md.shape[1]
                t0 = ci * MGT
                tn = min(MGT, nt - t0)
                mt = mpool.tile([128, MGT, D], fp8, tag="m" + which)
                nc.sync.dma_start(out=mt[:, :tn, :], in_=md[:, t0:t0 + tn, :])
                tiles[ci] = mt

            posA = np.concatenate([[0], np.cumsum(tA)]).astype(int)
            posB = np.concatenate([[0], np.cumsum(tB)]).astype(int)

            # pass A: h1-sourced edges -> aggA tiles (SBUF)
            aggsA = []
            b0 = 0
            while b0 < NBLK:
                gn = min(4, NBLK - b0)
                pa = pa_pool.tile([D, 512], f32, tag="pa")
                for qb in range(gn):
                    b = b0 + qb
                    kA = int(tA[b])
                    for j in range(kA):
                        g = posA[b] + j
                        ensure_chunk("A", g // p.GT)
                        ensure_mchunk("A", g // MGT)
                        nc.tensor.matmul(pa[:, qb * 128:(qb + 1) * 128],
                                         lhsT=gA_tiles[g // p.GT][:, g % p.GT, :],
                                         rhs=mA_tiles[g // MGT][:, g % MGT, :],
                                         start=(j == 0), stop=(j == kA - 1))
                aggA = aggApool.tile([D, 512], bf16, tag="aggA")
                nc.scalar.activation(aggA[:], pa[:], AF.Copy)
                aggsA.append(aggA)
                b0 += gn

            # pass B: h2-sourced edges; fuse the final linears per group
            b0 = 0
            while b0 < NBLK:
                gn = min(4, NBLK - b0)
                gi = b0 // 4
                pa = pa_pool.tile([D, 512], f32, tag="pa")
                for qb in range(gn):
                    b = b0 + qb
                    kB = int(tB[b])
                    for j in range(kB):
                        g = posB[b] + j
                        ensure_chunk("B", g // p.GT)
                        ensure_mchunk("B", g // MGT)
                        nc.tensor.matmul(pa[:, qb * 128:(qb + 1) * 128],
                                         lhsT=gB_tiles[g // p.GT][:, g % p.GT, :],
                                         rhs=mB_tiles[g // MGT][:, g % MGT, :],
                                         start=(j == 0), stop=(j == kB - 1))
                aggB = aggBpool.tile([D, 512], bf16, tag="aggB")
                nc.scalar.activation(aggB[:], pa[:], AF.Copy)

                w = min(512, NB - b0 * 128)
                po = po_pool.tile([D, 512], f32, tag="po")
                nc.tensor.matmul(po[:, :w], lhsT=WnW[:],
                                 rhs=xT_own[:, b0 * 128:b0 * 128 + w],
                                 start=True, stop=False)
                nc.tensor.matmul(po[:, :w], lhsT=WgW[:], rhs=aggsA[gi][:, :w],
                                 start=False, stop=False)
                nc.tensor.matmul(po[:, :w], lhsT=WgW[:], rhs=aggB[:, :w],
                                 start=False, stop=True)
                ot = opool.tile([D, 512], f32, tag="ot")
                nc.scalar.activation(ot[:, :w], po[:, :w], AF.Tanh,
                                     bias=biasT[:, 0:1])
                nc.sync.dma_start(out=outT_d[:, b0 * 128:b0 * 128 + w],
                                  in_=ot[:, :w])
                b0 += gn

    nc.compile()
    return nc


# ---------------------------------------------------------------- host entry


def _host_prep(p: P, x, edge_index, Wn_w, Wn_b, Wg_w, Wg_b, Wa_w, Wa_b):
    x = np.asarray(x, np.float32)
    xT = np.ascontiguousarray(x.T).astype(BF16)
    tA, tB, LA, LB, LAg, LBg, per_core = prep_edges(
        p, np.asarray(edge_index)[0], np.asarray(edge_index)[1])

    shared = {
        "xT": xT,
        "WaW": np.asarray(Wa_w, np.float32).astype(BF16),
        "WnW": np.asarray(Wn_w, np.float32).astype(BF16),
        "WgW": np.asarray(Wg_w, np.float32).astype(BF16),
        "WaB": np.asarray(Wa_b, np.float32).reshape(p.D, 1),
        "biasT": (np.asarray(Wn_b, np.float32)
                  + np.asarray(Wg_b, np.float32)).reshape(p.D, 1),
        "ident": np.eye(p.D, dtype=np.float32).astype(BF16),
    }
    in_maps = []
    for k in range(p.NCORES):
        m = dict(shared)
        m["xT_own"] = np.ascontiguousarray(xT[:, k * p.NB:(k + 1) * p.NB])
        pc = per_core[k]
        m["idxA"], m["idxB"] = pc["idxA"], pc["idxB"]
        m["MA"], m["MB"] = pc["MA"], pc["MB"]
        in_maps.append(m)
    return tA, tB, LA, LB, LAg, LBg, in_maps


TRACE = False      # set True (e.g. from test.py) to capture an NTFF profile
LAST = None        # last BassKernelResults, for profiling/inspection


def kernel(**inputs) -> np.ndarray:
    global LAST
    from concourse import bass_utils
    bass_utils.upload_artifacts = lambda tmpdir: "local://" + tmpdir

    p = P()
    tA, tB, LA, LB, LAg, LBg, in_maps = _host_prep(p, **inputs)
    nc = build(p, tA, tB, LA, LB, LAg, LBg)
    kw = dict(trace=True, trace_cores=list(range(p.NCORES))) if TRACE else {}
    res = bass_utils.run_bass_kernel_spmd(
        nc, in_maps, core_ids=list(range(p.NCORES)), **kw)
    LAST = res
    out = np.concatenate(
        [res.results[k]["outT"].T for k in range(p.NCORES)], axis=0)
    return np.ascontiguousarray(out).astype(np.float32)
